# revision 41
# baseline (speedup 1.0000x reference)
"""CPC NCE loss kernel for Trainium2, 8 NeuronCores — fp8 v2.

Sharding: 224 (i,k,j) NCE combos -> 28 per core = 14 chunks of 128 rows
(row = (j, b)).  Per core the 28 combos form 3 full (i,k) pairs (8 j's
= 512 rows) + 1 half pair (4 j's = 256 rows); a pair shares one Wk so
its linear layer runs with FD=512.

All matmuls run in fp8 e4m3 with DoubleRow perf mode where K >= 256
(2 fp8 weights per PE cell -> 2x throughput).  W is pre-scaled x16 and
Z (negatives + positives) by 1/16 so quantization keeps 3 significant
mantissa bits; the products are exact-scale.  Measured end-to-end
rel err ~1.3e-3 (gate 2e-2).

Per chunk (128 rows):
  zh^T = Wk^T.T @ C^T    (PE fp8 DR into PSUM; ACT Identity cast adds
                          the per-feature bias and emits fp8 zh)
  pos  = diag(zh @ Zpos^T)  (PE fp8 + DVE eye-mask stt accumulate)
  raw  = zh @ Zneg       (PE fp8 DR, 16 MMs of N=512, g-major so the
                          stationary operand is reused across 4 MMs;
                          Zneg columns n = (h*8+w)*64 + b)
  E    = exp(raw - 45)   (ACT reads the four 1024-wide PSUM groups
                          directly, dumps f32 exps to SBUF, no
                          accumulator -> no READ_ACCUMULATOR ops)
  S    = sum E*m01       (one fused 4096-wide DVE stt with accumulate;
                          m01 zeroes the 64 self-batch columns
                          n%64 == r%64)
Tail (batched over all 14 chunks): Epos=exp(pos-45), L=ln(Epos+S),
nce = (pos-45) - L.  Host sums -mean in f64.

Schedule: the next block's linear layer is spread one ec-piece per
chunk (casts on ACT), block inputs prefetched 2 blocks ahead, zn
loaded in column quarters so chunk 0 starts as data lands.  A warm-up
burst of 32 N=128 matmuls flips the PE HAM clock-gate to 2.4 GHz
before real compute.  The last two chunks interleave their 2048-wide
dump halves (split stt sums) to shorten the pipeline drain.
"""

import numpy as np
import ml_dtypes

import concourse.bass as bass
import concourse.tile as tile
from concourse import mybir
from concourse.vector_clock import ScopedClock
from concourse.bass_utils import run_bass_kernel_spmd

B, D, H, W = 64, 512, 8, 8
NCORES = 8
NCHUNKS = 14          # chunks per core (128 rows each)
NBLK = 4              # mm1 blocks per core: 3 full pairs + 1 half pair
BLK_R = [512, 512, 512, 256]   # rows per block
NQ = 4                # 1024-wide column quarters (PSUM groups) per chunk
M_SHIFT = 45.0
WSCALE = 16.0

F8 = mybir.dt.float8e4
F32 = mybir.dt.float32
BF = mybir.dt.bfloat16
NPF8 = ml_dtypes.float8_e4m3
BF16 = ml_dtypes.bfloat16

LAST_RESULTS = None
_cache = {}


def _split_multi_waits(nc):
    """walrus in this container accepts at most ONE sync wait per
    instruction; hoist extra waits onto preceding same-engine NOPs."""
    k = 0
    for f in nc.m.functions:
        for bb in f.blocks:
            newlist = []
            changed = False
            for inst in bb.instructions:
                si = inst.sync_info
                if si is not None and si.on_wait and len(si.on_wait) > 1:
                    waits = list(si.on_wait)
                    for w in waits[:-1]:
                        nop = mybir.InstNoOp(name=f"I-wsplit-{k}", ins=[], outs=[])
                        k += 1
                        nop.engine = inst.engine
                        nop.sync_info = mybir.SyncInfo(on_wait=[w], on_update=[])
                        newlist.append(nop)
                    inst.sync_info = mybir.SyncInfo(
                        on_wait=[waits[-1]], on_update=list(si.on_update or [])
                    )
                    changed = True
                newlist.append(inst)
            if changed:
                bb.instructions = newlist


class _TileContext(tile.TileContext):
    """Tail drain variant that keeps <=1 sem wait per instruction."""

    def _drain_and_barrier(self, tick_clock, wait_clock):
        nc = self.nc
        probe = nc.sync.nop(nofuse=True)
        wait_clock.add_sem_waits(
            probe.ins, ScopedClock({None: tick_clock.global_clock})
        )
        si = probe.ins.sync_info
        if si is not None and si.on_wait and len(si.on_wait) > 1:
            waits = list(si.on_wait)
            probe.ins.sync_info = mybir.SyncInfo(
                on_wait=waits[:1], on_update=list(si.on_update or [])
            )
            for w in waits[1:]:
                n2 = nc.sync.nop(nofuse=True)
                n2.ins.sync_info = mybir.SyncInfo(on_wait=[w], on_update=[])
        nc.sync.drain()
        nc.all_engine_barrier()
        assert self.sems is not None
        popped = nc._tile_sem_poison_stack.pop()
        assert popped is self._sem_poison
        nc.clear_and_free_semaphores(list(self.sems.allocated().values()))


def _build_module():
    nc = bass.Bass("TRN2", target_bir_lowering=False, debug=False)
    ap = {}
    # zn[p, q, g, i, c]: Zneg[d, n] with d = 256g+128i+p, n = 1024q + c
    ap["zn"] = nc.dram_tensor("zn", [128, NQ, 2, 2, 1024], F8, kind="ExternalInput").ap()
    # wdr[blk, p, g2, i2, ec, f] = Wk[k][128ec+f, 256g2+128i2+p] * WSCALE
    ap["wdr"] = nc.dram_tensor("wdr", [NBLK, 128, 2, 2, 4, 128], F8, kind="ExternalInput").ap()
    # cdr[blk, p, g2, i2, r] = C[b, 256g2+128i2+p, i_pair, j]
    ap["cdr"] = nc.dram_tensor("cdr", [NBLK, 128, 2, 2, 512], F8, kind="ExternalInput").ap()
    # bgc[blk, f, ec] = bk[k][128ec+f]
    ap["bgc"] = nc.dram_tensor("bgc", [NBLK, 128, 4], F32, kind="ExternalInput").ap()
    # zpc[t, p, ec, c] = Z[b_c, 128ec+p, k_t, j_c]  (positive targets)
    ap["zpc"] = nc.dram_tensor("zpc", [NCHUNKS, 128, 4, 128], F8, kind="ExternalInput").ap()
    ap["m64"] = nc.dram_tensor("m64", [128, 64], BF, kind="ExternalInput").ap()
    ap["eye"] = nc.dram_tensor("eye", [128, 128], F32, kind="ExternalInput").ap()
    out_ap = nc.dram_tensor("out", [128, NCHUNKS], F32, kind="ExternalOutput").ap()

    Exp = mybir.ActivationFunctionType.Exp
    Ln = mybir.ActivationFunctionType.Ln
    Ident = mybir.ActivationFunctionType.Identity
    Add = mybir.AluOpType.add
    Mult = mybir.AluOpType.mult
    Sub = mybir.AluOpType.subtract
    DR = mybir.MatmulPerfMode.DoubleRow

    # chunk -> (block, row slice within block)
    chunk_map = []
    for blk in range(NBLK):
        for t in range(BLK_R[blk] // 128):
            chunk_map.append((blk, t))
    assert len(chunk_map) == NCHUNKS

    with _TileContext(nc) as tc:
        with (
            tc.tile_pool(name="consts", bufs=1) as consts,
            tc.tile_pool(name="wpool", bufs=2) as wpool,
            tc.tile_pool(name="cpool", bufs=2) as cpool,
            tc.tile_pool(name="bgpool", bufs=2) as bgpool,
            tc.tile_pool(name="zhpool", bufs=2) as zhpool,
            tc.tile_pool(name="zppool", bufs=4) as zppool,
            tc.tile_pool(name="dumpp", bufs=3) as dumpp,
            tc.tile_pool(name="trash", bufs=1) as trash,
            tc.tile_pool(name="ps_raw", bufs=3, space="PSUM") as ps_raw,
            tc.tile_pool(name="ps_zh", bufs=2, space="PSUM") as ps_zh,
        ):
            def load_block(blk):
                wt = wpool.tile([128, 2, 2, 4, 128], F8)
                nc.sync.dma_start(wt[:], ap["wdr"][blk])
                ct = cpool.tile([128, 2, 2, 512], F8)
                nc.sync.dma_start(ct[:], ap["cdr"][blk])
                bt = bgpool.tile([128, 4], F32)
                nc.sync.dma_start(bt[:], ap["bgc"][blk])
                return wt, ct, bt

            def load_zp(t):
                zp = zppool.tile([128, 4, 128], F8)
                nc.sync.dma_start(zp[:], ap["zpc"][t])
                return zp

            def mm1_alloc(blk):
                R = BLK_R[blk]
                zh = zhpool.tile([128, 4, R], F8)
                return zh

            def mm1_step(blk, zh, wt, ct, bt, ec):
                """One ec piece of a block's linear layer: 2 DR matmuls +
                ACT bias-add-cast to fp8."""
                R = BLK_R[blk]
                zh_ps = ps_zh.tile([128, 512], F32, tag="zh")
                for g2 in range(2):
                    nc.tensor.matmul(
                        zh_ps[:, 0:R],
                        wt[:, g2, :, ec, :],
                        ct[:, g2, :, 0:R],
                        start=(g2 == 0),
                        stop=(g2 == 1),
                        perf_mode=DR,
                    )
                if ec < 2:
                    nc.scalar.activation(
                        zh[:, ec, :], zh_ps[:, 0:R], Ident,
                        bias=bt[:, ec:ec + 1], scale=1.0,
                    )
                else:
                    nc.vector.tensor_scalar(
                        out=zh[:, ec, :], in0=zh_ps[:, 0:R],
                        scalar1=bt[:, ec:ec + 1], scalar2=None,
                        op0=mybir.AluOpType.add,
                    )

            def mm1(blk, wt, ct, bt):
                zh = mm1_alloc(blk)
                for ec in range(4):
                    mm1_step(blk, zh, wt, ct, bt, ec)
                return zh

            # ---- constants + first loads (zn quarters prioritized) ----
            blk_in = [None] * NBLK
            blk_in[0] = load_block(0)
            zn_t = consts.tile([128, NQ, 2, 2, 1024], F8)
            nc.sync.dma_start(zn_t[:, 0], ap["zn"][:, 0])
            m01_t = consts.tile([128, 4096], BF)
            eye_t = consts.tile([128, 128], F32)
            nc.sync.dma_start(m01_t[:, 0:64], ap["m64"][:])
            nc.sync.dma_start(eye_t[:], ap["eye"][:])
            zps = load_zp(0)
            for q in range(1, NQ):
                nc.sync.dma_start(zn_t[:, q], ap["zn"][:, q])
            blk_in[1] = load_block(1)
            # replicate the 64-wide self-mask across all 64 hw blocks
            w_ = 64
            while w_ < 4096:
                nc.vector.tensor_copy(m01_t[:, w_:2 * w_], m01_t[:, 0:w_])
                w_ *= 2

            negM = consts.tile([128, 1], F32)
            nc.vector.memset(negM[:], -M_SHIFT)
            pos_sb = consts.tile([128, NCHUNKS], F32)
            S1 = consts.tile([128, NCHUNKS], F32)
            S1b = consts.tile([128, 2], F32)
            out_t = consts.tile([128, NCHUNKS], F32)

            # ---- PE warm-up spin: tiny matmuls while DMAs land (keeps the
            # HAM clock-gate warm until real compute arrives)
            wspin = consts.tile([128, 128], BF)
            nc.vector.memset(wspin[:], 0.0)
            spin_ps = ps_raw.tile([128, 1024], F32, tag="raw")
            for _ in range(32):
                nc.tensor.matmul(spin_ps[:, 0:128], wspin[:], wspin[:],
                                 start=True, stop=True)

            # ---- main loop ----
            zh_cur = mm1(0, *blk_in[0])
            zh_next = None
            for t, (blk, tb) in enumerate(chunk_map[:NCHUNKS - 2]):
                # start of a block: kick next block's input DMA
                if tb == 0:
                    if blk + 2 < NBLK:
                        blk_in[blk + 2] = load_block(blk + 2)
                    if blk + 1 < NBLK:
                        zh_next = mm1_alloc(blk + 1)
                if t + 1 < NCHUNKS:
                    zp_next = load_zp(t + 1)

                rs = slice(tb * 128, (tb + 1) * 128)
                dump = dumpp.tile([128, 4096], F32)

                # pos = diag(zh @ Zpos^T)
                pos_ps = ps_zh.tile([128, 512], F32, tag="zh")
                for ec in range(4):
                    nc.tensor.matmul(
                        pos_ps[:, 0:128], zh_cur[:, ec, rs], zps[:, ec, :],
                        start=(ec == 0), stop=(ec == 3),
                    )
                dsc = trash.tile([128, 128], F32, tag="dsc")
                nc.vector.scalar_tensor_tensor(
                    out=dsc[:], in0=pos_ps[:, 0:128], scalar=1.0, in1=eye_t[:],
                    op0=Mult, op1=Mult, accum_out=pos_sb[:, t:t + 1],
                )

                # raw = zh @ Zneg: group-pair blocks, g-major inside so the
                # DR stationary operand is reused across 4 consecutive MMs
                for qq in range(2):
                    rps = [ps_raw.tile([128, 1024], F32, tag="raw",
                                       name=f"raw_ps_{t}_{qq}_{qi}")
                           for qi in range(2)]
                    for g in range(2):
                        for qi in range(2):
                            for half in range(2):
                                cs = slice(half * 512, (half + 1) * 512)
                                nc.tensor.matmul(
                                    rps[qi][:, cs],
                                    zh_cur[:, 2 * g:2 * g + 2, rs],
                                    zn_t[:, 2 * qq + qi, g, :, cs],
                                    start=(g == 0),
                                    stop=(g == 1),
                                    perf_mode=DR,
                                )
                    for qi in range(2):
                        q = 2 * qq + qi
                        nc.scalar.activation(
                            dump[:, q * 1024:(q + 1) * 1024], rps[qi][:], Exp,
                            bias=negM[:, 0:1], scale=1.0,
                        )
                if t < NCHUNKS - 3:
                    # mask is 64-periodic: fold the dump in half with a
                    # DMA accumulate (idle engine), halving the DVE sum
                    nc.gpsimd.dma_start(dump[:, 0:2048], dump[:, 2048:4096],
                                         accum_op=Add)
                    tdve = trash.tile([128, 2048], BF, tag="tdve",
                                      name=f"tdve_f{t}")
                    nc.vector.scalar_tensor_tensor(
                        out=tdve[:], in0=dump[:, 0:2048], scalar=1.0,
                        in1=m01_t[:, 0:2048], op0=Mult, op1=Mult,
                        accum_out=S1[:, t:t + 1],
                    )
                else:
                    tdve = trash.tile([128, 4096], BF, tag="tdve")
                    nc.vector.scalar_tensor_tensor(
                        out=tdve[:], in0=dump[:], scalar=1.0,
                        in1=m01_t[:], op0=Mult, op1=Mult,
                        accum_out=S1[:, t:t + 1],
                    )

                # one ec piece of the next block's linear layer per chunk
                # (issued at chunk end so chunk 0 never waits on block 1)
                if blk + 1 < NBLK and tb < 3:
                    if tb == 0:
                        mm1_step(blk + 1, zh_next, *blk_in[blk + 1], 0)
                        mm1_step(blk + 1, zh_next, *blk_in[blk + 1], 1)
                    else:
                        mm1_step(blk + 1, zh_next, *blk_in[blk + 1], tb + 1)

                if t + 1 < NCHUNKS:
                    zps = zp_next
                if tb == BLK_R[blk] // 128 - 1:
                    zh_cur = zh_next

            # ---- last two chunks: interleave the two 2048-halves of
            # both chunks so exps/sums overlap the final matmuls ----
            zp13 = load_zp(NCHUNKS - 1)
            lzh = zh_cur
            ldumps = [dumpp.tile([128, 4096], F32, name=f"dump_l{i}")
                      for i in range(2)]
            for i, lt in enumerate((NCHUNKS - 2, NCHUNKS - 1)):
                lrs = slice(i * 128, (i + 1) * 128)
                lzp = zps if i == 0 else zp13
                pos_ps = ps_zh.tile([128, 512], F32, tag="zh",
                                    name=f"pos_ps_l{i}")
                for ec in range(4):
                    nc.tensor.matmul(
                        pos_ps[:, 0:128], lzh[:, ec, lrs], lzp[:, ec, :],
                        start=(ec == 0), stop=(ec == 3),
                    )
                dsc = trash.tile([128, 128], F32, tag="dsc",
                                 name=f"dsc_l{i}")
                nc.vector.scalar_tensor_tensor(
                    out=dsc[:], in0=pos_ps[:, 0:128], scalar=1.0,
                    in1=eye_t[:], op0=Mult, op1=Mult,
                    accum_out=pos_sb[:, lt:lt + 1],
                )
            for qq in range(2):
                for i, lt in enumerate((NCHUNKS - 2, NCHUNKS - 1)):
                    lrs = slice(i * 128, (i + 1) * 128)
                    rps = [ps_raw.tile([128, 1024], F32, tag="raw",
                                       name=f"raw_ps_l{i}_{qq}_{qi}")
                           for qi in range(2)]
                    for g in range(2):
                        for qi in range(2):
                            for half in range(2):
                                cs = slice(half * 512, (half + 1) * 512)
                                nc.tensor.matmul(
                                    rps[qi][:, cs],
                                    lzh[:, 2 * g:2 * g + 2, lrs],
                                    zn_t[:, 2 * qq + qi, g, :, cs],
                                    start=(g == 0),
                                    stop=(g == 1),
                                    perf_mode=DR,
                                )
                    for qi in range(2):
                        q = 2 * qq + qi
                        nc.scalar.activation(
                            ldumps[i][:, q * 1024:(q + 1) * 1024],
                            rps[qi][:], Exp, bias=negM[:, 0:1], scale=1.0,
                        )
                    hs = slice(qq * 2048, (qq + 1) * 2048)
                    tdve = trash.tile([128, 2048], BF, tag="tdve",
                                      name=f"tdve_l{i}_{qq}")
                    nc.vector.scalar_tensor_tensor(
                        out=tdve[:], in0=ldumps[i][:, hs], scalar=1.0,
                        in1=m01_t[:, hs], op0=Mult, op1=Mult,
                        accum_out=(S1[:, lt:lt + 1] if qq == 0
                                   else S1b[:, i:i + 1]),
                    )

            # ---- batched tail math over all 14 chunks ----
            nc.vector.tensor_tensor(
                out=S1[:, NCHUNKS - 2:NCHUNKS], in0=S1[:, NCHUNKS - 2:NCHUNKS],
                in1=S1b[:], op=Add)
            Epos = consts.tile([128, NCHUNKS], F32)
            nc.scalar.activation(Epos[:], pos_sb[:], Exp, bias=negM[:, 0:1])
            Tt = consts.tile([128, NCHUNKS], F32)
            nc.vector.tensor_tensor(out=Tt[:], in0=Epos[:], in1=S1[:], op=Add)
            Lt = consts.tile([128, NCHUNKS], F32)
            nc.scalar.activation(Lt[:], Tt[:], Ln)
            nc.vector.scalar_tensor_tensor(
                out=out_t[:], in0=pos_sb[:], scalar=-M_SHIFT, in1=Lt[:],
                op0=Add, op1=Sub,
            )
            nc.sync.dma_start(out_ap[:], out_t[:])

    _split_multi_waits(nc)
    return nc


def _core_blocks(c):
    """Per-core mm1 blocks: 3 full pairs + 1 half pair, uniform program.

    Global combo g = pair*8 + j; core c owns combos [28c, 28c+28).
    Returns ([(pair, j_start, n_j)] * 4) with the half block last.
    """
    if c % 2 == 0:
        p0 = (7 * c) // 2
        return [(p0, 0, 8), (p0 + 1, 0, 8), (p0 + 2, 0, 8), (p0 + 3, 0, 4)]
    else:
        p0 = (7 * c) // 2
        return [(p0 + 1, 0, 8), (p0 + 2, 0, 8), (p0 + 3, 0, 8), (p0, 4, 4)]


def _prep_inputs(Z, C, Wk, bk):
    ii, kk = np.triu_indices(H, 1)

    # Zneg columns n = (h*8+w)*64 + b, rows d; DR layout [p, q, g, i, c]
    Znegs = Z.transpose(1, 2, 3, 0).reshape(D, 4096) / WSCALE
    zn = Znegs.reshape(2, 2, 128, 4, 1024).transpose(2, 3, 0, 1, 4)
    zn = np.ascontiguousarray(zn).astype(NPF8)

    # W^T in DR layout per pair: wdrp[pair][p, g2, i2, ec, f]
    # = Wk[k-1][128ec+f, 256g2+128i2+p] * WSCALE
    WT = (Wk.transpose(0, 2, 1) * WSCALE)  # [pair_k][d, e]
    wdr_all = WT.reshape(7, 2, 2, 128, 4, 128).transpose(0, 3, 1, 2, 4, 5)
    wdr_all = np.ascontiguousarray(wdr_all).astype(NPF8)  # [7, p, g2, i2, ec, f]

    # C^T per (i, j): [d, b] ; zpos per (k, j): [d, b]
    Ctr = C.transpose(2, 3, 1, 0)  # [i, j, d, b]
    Ztr = Z.transpose(2, 3, 1, 0)  # [h, w, d, b]

    m64 = (np.arange(64)[None, :] != np.arange(128)[:, None] % 64)
    m64 = m64.astype(BF16)
    eye = np.eye(128, dtype=np.float32)

    in_maps = []
    for c in range(NCORES):
        blocks = _core_blocks(c)
        wdr = np.empty((NBLK, 128, 2, 2, 4, 128), NPF8)
        cdr = np.zeros((NBLK, 128, 2, 2, 512), NPF8)
        bgc = np.empty((NBLK, 128, 4), np.float32)
        zpc = np.empty((NCHUNKS, 128, 4, 128), NPF8)
        t = 0
        for blk, (pair, j0, nj) in enumerate(blocks):
            i_, k_ = int(ii[pair]), int(kk[pair])
            wdr[blk] = wdr_all[k_ - 1]
            bgc[blk] = bk[k_ - 1].reshape(4, 128).T * WSCALE
            # cdr[p, g2, i2, r] with r = (j-j0)*64 + b, d = 256g2+128i2+p
            cblk = Ctr[i_, j0:j0 + nj]          # [nj, d, b]
            cblk = cblk.transpose(1, 0, 2).reshape(2, 2, 128, nj * 64)
            cdr[blk, :, :, :, 0:nj * 64] = cblk.transpose(2, 0, 1, 3).astype(NPF8)
            # chunks of this block
            for tb in range(nj // 2):
                ja, jb_ = j0 + 2 * tb, j0 + 2 * tb + 1
                zp = np.concatenate(
                    [Ztr[k_, ja], Ztr[k_, jb_]], axis=1) / WSCALE  # [d, 128]
                zpc[t] = zp.reshape(4, 128, 128).transpose(1, 0, 2).astype(NPF8)
                t += 1
        assert t == NCHUNKS
        in_maps.append({
            "zn": zn, "wdr": wdr, "cdr": cdr, "bgc": bgc, "zpc": zpc,
            "m64": m64, "eye": eye,
        })
    return in_maps


def _chunk_info(c):
    """Host-side (k, j0) per chunk for each core (for testing)."""
    ii, kk = np.triu_indices(H, 1)
    info = []
    for pair, j0, nj in _core_blocks(c):
        for tb in range(nj // 2):
            info.append((int(ii[pair]), int(kk[pair]), j0 + 2 * tb))
    return info


def kernel(Z, C, Wk, bk):
    global LAST_RESULTS
    Z = np.asarray(Z, np.float32)
    C = np.asarray(C, np.float32)
    Wk = np.asarray(Wk, np.float32)
    bk = np.asarray(bk, np.float32)

    if "nc" not in _cache:
        _cache["nc"] = _build_module()
    nc = _cache["nc"]

    in_maps = _prep_inputs(Z, C, Wk, bk)
    res = run_bass_kernel_spmd(nc, in_maps, core_ids=list(range(NCORES)))
    LAST_RESULTS = res
    total = np.float64(0.0)
    for c in range(NCORES):
        total += np.sum(res.results[c]["out"].astype(np.float64))
    loss = -(total / (NCORES * NCHUNKS * 128))
    return np.array(loss, dtype=np.float32)


# revision 42
# speedup vs baseline: 1.1773x; 1.1773x over previous
"""CPC NCE loss kernel for Trainium2, 8 NeuronCores — fp8 v2.

Sharding: 224 (i,k,j) NCE combos -> 28 per core = 14 chunks of 128 rows
(row = (j, b)).  Per core the 28 combos form 3 full (i,k) pairs (8 j's
= 512 rows) + 1 half pair (4 j's = 256 rows); a pair shares one Wk so
its linear layer runs with FD=512.

All matmuls run in fp8 e4m3 with DoubleRow perf mode where K >= 256
(2 fp8 weights per PE cell -> 2x throughput).  W is pre-scaled x16 and
Z (negatives + positives) by 1/16 so quantization keeps 3 significant
mantissa bits; the products are exact-scale.  Measured end-to-end
rel err ~1.3e-3 (gate 2e-2).

Per chunk (128 rows):
  zh^T = Wk^T.T @ C^T    (PE fp8 DR into PSUM; ACT Identity cast adds
                          the per-feature bias and emits fp8 zh)
  pos  = diag(zh @ Zpos^T)  (PE fp8 + DVE eye-mask stt accumulate)
  raw  = zh @ Zneg       (PE fp8 DR, 16 MMs of N=512, g-major so the
                          stationary operand is reused across 4 MMs;
                          Zneg columns n = (h*8+w)*64 + b)
  E    = exp(raw - 45)   (ACT reads the four 1024-wide PSUM groups
                          directly, dumps f32 exps to SBUF, no
                          accumulator -> no READ_ACCUMULATOR ops)
  S    = sum E*m01       (one fused 4096-wide DVE stt with accumulate;
                          m01 zeroes the 64 self-batch columns
                          n%64 == r%64)
Tail (batched over all 14 chunks): Epos=exp(pos-45), L=ln(Epos+S),
nce = (pos-45) - L.  Host sums -mean in f64.

Schedule: the next block's linear layer is spread one ec-piece per
chunk (casts on ACT), block inputs prefetched 2 blocks ahead, zn
loaded in column quarters so chunk 0 starts as data lands.  A warm-up
burst of 32 N=128 matmuls flips the PE HAM clock-gate to 2.4 GHz
before real compute.  The last two chunks interleave their 2048-wide
dump halves (split stt sums) to shorten the pipeline drain.
"""

import numpy as np
import ml_dtypes

import concourse.bass as bass
import concourse.tile as tile
from concourse import mybir
from concourse.vector_clock import ScopedClock
from concourse.bass_utils import run_bass_kernel_spmd

B, D, H, W = 64, 512, 8, 8
NCORES = 8
NCHUNKS = 14          # chunks per core (128 rows each)
NBLK = 4              # mm1 blocks per core: 3 full pairs + 1 half pair
BLK_R = [512, 512, 512, 256]   # rows per block
NQ = 4                # 1024-wide column quarters (PSUM groups) per chunk
M_SHIFT = 45.0
WSCALE = 16.0

F8 = mybir.dt.float8e4
F32 = mybir.dt.float32
BF = mybir.dt.bfloat16
NPF8 = ml_dtypes.float8_e4m3
BF16 = ml_dtypes.bfloat16

LAST_RESULTS = None
_cache = {}


def _split_multi_waits(nc):
    """walrus in this container accepts at most ONE sync wait per
    instruction; hoist extra waits onto preceding same-engine NOPs."""
    k = 0
    for f in nc.m.functions:
        for bb in f.blocks:
            newlist = []
            changed = False
            for inst in bb.instructions:
                si = inst.sync_info
                if si is not None and si.on_wait and len(si.on_wait) > 1:
                    waits = list(si.on_wait)
                    for w in waits[:-1]:
                        nop = mybir.InstNoOp(name=f"I-wsplit-{k}", ins=[], outs=[])
                        k += 1
                        nop.engine = inst.engine
                        nop.sync_info = mybir.SyncInfo(on_wait=[w], on_update=[])
                        newlist.append(nop)
                    inst.sync_info = mybir.SyncInfo(
                        on_wait=[waits[-1]], on_update=list(si.on_update or [])
                    )
                    changed = True
                newlist.append(inst)
            if changed:
                bb.instructions = newlist


class _TileContext(tile.TileContext):
    """Tail drain variant that keeps <=1 sem wait per instruction."""

    def _drain_and_barrier(self, tick_clock, wait_clock):
        nc = self.nc
        probe = nc.sync.nop(nofuse=True)
        wait_clock.add_sem_waits(
            probe.ins, ScopedClock({None: tick_clock.global_clock})
        )
        si = probe.ins.sync_info
        if si is not None and si.on_wait and len(si.on_wait) > 1:
            waits = list(si.on_wait)
            probe.ins.sync_info = mybir.SyncInfo(
                on_wait=waits[:1], on_update=list(si.on_update or [])
            )
            for w in waits[1:]:
                n2 = nc.sync.nop(nofuse=True)
                n2.ins.sync_info = mybir.SyncInfo(on_wait=[w], on_update=[])
        nc.sync.drain()
        nc.all_engine_barrier()
        assert self.sems is not None
        popped = nc._tile_sem_poison_stack.pop()
        assert popped is self._sem_poison
        nc.clear_and_free_semaphores(list(self.sems.allocated().values()))


def _build_module():
    nc = bass.Bass("TRN2", target_bir_lowering=False, debug=False)
    ap = {}
    # zn[p, q, g, i, c]: Zneg[d, n] with d = 256g+128i+p, n = 1024q + c
    ap["zn"] = nc.dram_tensor("zn", [128, NQ, 2, 2, 1024], F8, kind="ExternalInput").ap()
    # wdr[blk, p, g2, i2, ec, f] = Wk[k][128ec+f, 256g2+128i2+p] * WSCALE
    ap["wdr"] = nc.dram_tensor("wdr", [NBLK, 128, 2, 2, 4, 128], F8, kind="ExternalInput").ap()
    # cdr[blk, p, g2, i2, r] = C[b, 256g2+128i2+p, i_pair, j]
    ap["cdr"] = nc.dram_tensor("cdr", [NBLK, 128, 2, 2, 512], F8, kind="ExternalInput").ap()
    # bgc[blk, f, ec] = bk[k][128ec+f]
    ap["bgc"] = nc.dram_tensor("bgc", [NBLK, 128, 4], F32, kind="ExternalInput").ap()
    # zpc[t, p, ec, c] = Z[b_c, 128ec+p, k_t, j_c]  (positive targets)
    ap["zpc"] = nc.dram_tensor("zpc", [NCHUNKS, 128, 4, 128], F8, kind="ExternalInput").ap()
    ap["m64"] = nc.dram_tensor("m64", [128, 64], BF, kind="ExternalInput").ap()
    ap["eye"] = nc.dram_tensor("eye", [128, 128], F32, kind="ExternalInput").ap()
    out_ap = nc.dram_tensor("out", [128, NCHUNKS], F32, kind="ExternalOutput").ap()

    Exp = mybir.ActivationFunctionType.Exp
    Ln = mybir.ActivationFunctionType.Ln
    Ident = mybir.ActivationFunctionType.Identity
    Add = mybir.AluOpType.add
    Mult = mybir.AluOpType.mult
    Sub = mybir.AluOpType.subtract
    DR = mybir.MatmulPerfMode.DoubleRow

    # chunk -> (block, row slice within block)
    chunk_map = []
    for blk in range(NBLK):
        for t in range(BLK_R[blk] // 128):
            chunk_map.append((blk, t))
    assert len(chunk_map) == NCHUNKS

    with _TileContext(nc) as tc:
        with (
            tc.tile_pool(name="consts", bufs=1) as consts,
            tc.tile_pool(name="wpool", bufs=2) as wpool,
            tc.tile_pool(name="cpool", bufs=2) as cpool,
            tc.tile_pool(name="bgpool", bufs=2) as bgpool,
            tc.tile_pool(name="zhpool", bufs=2) as zhpool,
            tc.tile_pool(name="zppool", bufs=4) as zppool,
            tc.tile_pool(name="dumpp", bufs=3) as dumpp,
            tc.tile_pool(name="trash", bufs=1) as trash,
            tc.tile_pool(name="ps_raw", bufs=3, space="PSUM") as ps_raw,
            tc.tile_pool(name="ps_zh", bufs=2, space="PSUM") as ps_zh,
        ):
            def load_block(blk):
                wt = wpool.tile([128, 2, 2, 4, 128], F8)
                nc.sync.dma_start(wt[:], ap["wdr"][blk])
                ct = cpool.tile([128, 2, 2, 512], F8)
                nc.sync.dma_start(ct[:], ap["cdr"][blk])
                bt = bgpool.tile([128, 4], F32)
                nc.sync.dma_start(bt[:], ap["bgc"][blk])
                return wt, ct, bt

            def load_zp(t):
                zp = zppool.tile([128, 4, 128], F8)
                nc.sync.dma_start(zp[:], ap["zpc"][t])
                return zp

            def mm1_alloc(blk):
                R = BLK_R[blk]
                zh = zhpool.tile([128, 4, R], F8)
                return zh

            def mm1_step(blk, zh, wt, ct, bt, ec):
                """One ec piece of a block's linear layer: 2 DR matmuls +
                ACT bias-add-cast to fp8."""
                R = BLK_R[blk]
                zh_ps = ps_zh.tile([128, 512], F32, tag="zh")
                for g2 in range(2):
                    nc.tensor.matmul(
                        zh_ps[:, 0:R],
                        wt[:, g2, :, ec, :],
                        ct[:, g2, :, 0:R],
                        start=(g2 == 0),
                        stop=(g2 == 1),
                        perf_mode=DR,
                    )
                nc.scalar.activation(
                    zh[:, ec, :], zh_ps[:, 0:R], Ident,
                    bias=bt[:, ec:ec + 1], scale=1.0,
                )

            def mm1(blk, wt, ct, bt):
                zh = mm1_alloc(blk)
                for ec in range(4):
                    mm1_step(blk, zh, wt, ct, bt, ec)
                return zh

            # ---- constants + first loads (zn quarters prioritized) ----
            blk_in = [None] * NBLK
            blk_in[0] = load_block(0)
            zn_t = consts.tile([128, NQ, 2, 2, 1024], F8)
            nc.sync.dma_start(zn_t[:, 0], ap["zn"][:, 0])
            m01_t = consts.tile([128, 4096], BF)
            eye_t = consts.tile([128, 128], F32)
            nc.sync.dma_start(m01_t[:, 0:64], ap["m64"][:])
            nc.sync.dma_start(eye_t[:], ap["eye"][:])
            zps = load_zp(0)
            for q in range(1, NQ):
                nc.sync.dma_start(zn_t[:, q], ap["zn"][:, q])
            blk_in[1] = load_block(1)
            # replicate the 64-wide self-mask across all 64 hw blocks
            w_ = 64
            while w_ < 4096:
                nc.vector.tensor_copy(m01_t[:, w_:2 * w_], m01_t[:, 0:w_])
                w_ *= 2

            negM = consts.tile([128, 1], F32)
            nc.vector.memset(negM[:], -M_SHIFT)
            pos_sb = consts.tile([128, NCHUNKS], F32)
            S1 = consts.tile([128, NCHUNKS], F32)
            S1b = consts.tile([128, 2], F32)
            out_t = consts.tile([128, NCHUNKS], F32)

            # ---- PE warm-up spin: tiny matmuls while DMAs land (keeps the
            # HAM clock-gate warm until real compute arrives)
            wspin = consts.tile([128, 128], BF)
            nc.vector.memset(wspin[:], 0.0)
            spin_ps = ps_raw.tile([128, 1024], F32, tag="raw")
            for _ in range(32):
                nc.tensor.matmul(spin_ps[:, 0:128], wspin[:], wspin[:],
                                 start=True, stop=True)

            # ---- main loop ----
            zh_cur = mm1(0, *blk_in[0])
            zh_next = None
            for t, (blk, tb) in enumerate(chunk_map[:NCHUNKS - 2]):
                # start of a block: kick next block's input DMA
                if tb == 0:
                    if blk + 2 < NBLK:
                        blk_in[blk + 2] = load_block(blk + 2)
                    if blk + 1 < NBLK:
                        zh_next = mm1_alloc(blk + 1)
                if t + 1 < NCHUNKS:
                    zp_next = load_zp(t + 1)

                rs = slice(tb * 128, (tb + 1) * 128)
                dump = dumpp.tile([128, 4096], F32)

                # pos = diag(zh @ Zpos^T)
                pos_ps = ps_zh.tile([128, 512], F32, tag="zh")
                for ec in range(4):
                    nc.tensor.matmul(
                        pos_ps[:, 0:128], zh_cur[:, ec, rs], zps[:, ec, :],
                        start=(ec == 0), stop=(ec == 3),
                    )
                dsc = trash.tile([128, 128], F32, tag="dsc")
                nc.vector.scalar_tensor_tensor(
                    out=dsc[:], in0=pos_ps[:, 0:128], scalar=1.0, in1=eye_t[:],
                    op0=Mult, op1=Mult, accum_out=pos_sb[:, t:t + 1],
                )

                # raw = zh @ Zneg: group-pair blocks, g-major inside so the
                # DR stationary operand is reused across 4 consecutive MMs
                for qq in range(2):
                    rps = [ps_raw.tile([128, 1024], F32, tag="raw",
                                       name=f"raw_ps_{t}_{qq}_{qi}")
                           for qi in range(2)]
                    for g in range(2):
                        for qi in range(2):
                            for half in range(2):
                                cs = slice(half * 512, (half + 1) * 512)
                                nc.tensor.matmul(
                                    rps[qi][:, cs],
                                    zh_cur[:, 2 * g:2 * g + 2, rs],
                                    zn_t[:, 2 * qq + qi, g, :, cs],
                                    start=(g == 0),
                                    stop=(g == 1),
                                    perf_mode=DR,
                                )
                    for qi in range(2):
                        q = 2 * qq + qi
                        nc.scalar.activation(
                            dump[:, q * 1024:(q + 1) * 1024], rps[qi][:], Exp,
                            bias=negM[:, 0:1], scale=1.0,
                        )
                tdve = trash.tile([128, 4096], BF, tag="tdve")
                nc.vector.scalar_tensor_tensor(
                    out=tdve[:], in0=dump[:], scalar=1.0,
                    in1=m01_t[:], op0=Mult, op1=Mult,
                    accum_out=S1[:, t:t + 1],
                )

                # one ec piece of the next block's linear layer per chunk
                # (issued at chunk end so chunk 0 never waits on block 1)
                if blk + 1 < NBLK and tb < 3:
                    if tb == 0:
                        mm1_step(blk + 1, zh_next, *blk_in[blk + 1], 0)
                        mm1_step(blk + 1, zh_next, *blk_in[blk + 1], 1)
                    else:
                        mm1_step(blk + 1, zh_next, *blk_in[blk + 1], tb + 1)

                if t + 1 < NCHUNKS:
                    zps = zp_next
                if tb == BLK_R[blk] // 128 - 1:
                    zh_cur = zh_next

            # ---- last two chunks: interleave the two 2048-halves of
            # both chunks so exps/sums overlap the final matmuls ----
            zp13 = load_zp(NCHUNKS - 1)
            lzh = zh_cur
            ldumps = [dumpp.tile([128, 4096], F32, name=f"dump_l{i}")
                      for i in range(2)]
            for i, lt in enumerate((NCHUNKS - 2, NCHUNKS - 1)):
                lrs = slice(i * 128, (i + 1) * 128)
                lzp = zps if i == 0 else zp13
                pos_ps = ps_zh.tile([128, 512], F32, tag="zh",
                                    name=f"pos_ps_l{i}")
                for ec in range(4):
                    nc.tensor.matmul(
                        pos_ps[:, 0:128], lzh[:, ec, lrs], lzp[:, ec, :],
                        start=(ec == 0), stop=(ec == 3),
                    )
                dsc = trash.tile([128, 128], F32, tag="dsc",
                                 name=f"dsc_l{i}")
                nc.vector.scalar_tensor_tensor(
                    out=dsc[:], in0=pos_ps[:, 0:128], scalar=1.0,
                    in1=eye_t[:], op0=Mult, op1=Mult,
                    accum_out=pos_sb[:, lt:lt + 1],
                )
            for qq in range(2):
                for i, lt in enumerate((NCHUNKS - 2, NCHUNKS - 1)):
                    lrs = slice(i * 128, (i + 1) * 128)
                    rps = [ps_raw.tile([128, 1024], F32, tag="raw",
                                       name=f"raw_ps_l{i}_{qq}_{qi}")
                           for qi in range(2)]
                    for g in range(2):
                        for qi in range(2):
                            for half in range(2):
                                cs = slice(half * 512, (half + 1) * 512)
                                nc.tensor.matmul(
                                    rps[qi][:, cs],
                                    lzh[:, 2 * g:2 * g + 2, lrs],
                                    zn_t[:, 2 * qq + qi, g, :, cs],
                                    start=(g == 0),
                                    stop=(g == 1),
                                    perf_mode=DR,
                                )
                    for qi in range(2):
                        q = 2 * qq + qi
                        nc.scalar.activation(
                            ldumps[i][:, q * 1024:(q + 1) * 1024],
                            rps[qi][:], Exp, bias=negM[:, 0:1], scale=1.0,
                        )
                    hs = slice(qq * 2048, (qq + 1) * 2048)
                    tdve = trash.tile([128, 2048], BF, tag="tdve",
                                      name=f"tdve_l{i}_{qq}")
                    nc.vector.scalar_tensor_tensor(
                        out=tdve[:], in0=ldumps[i][:, hs], scalar=1.0,
                        in1=m01_t[:, hs], op0=Mult, op1=Mult,
                        accum_out=(S1[:, lt:lt + 1] if qq == 0
                                   else S1b[:, i:i + 1]),
                    )

            # ---- batched tail math over all 14 chunks ----
            nc.vector.tensor_tensor(
                out=S1[:, NCHUNKS - 2:NCHUNKS], in0=S1[:, NCHUNKS - 2:NCHUNKS],
                in1=S1b[:], op=Add)
            Epos = consts.tile([128, NCHUNKS], F32)
            nc.scalar.activation(Epos[:], pos_sb[:], Exp, bias=negM[:, 0:1])
            Tt = consts.tile([128, NCHUNKS], F32)
            nc.vector.tensor_tensor(out=Tt[:], in0=Epos[:], in1=S1[:], op=Add)
            Lt = consts.tile([128, NCHUNKS], F32)
            nc.scalar.activation(Lt[:], Tt[:], Ln)
            nc.vector.scalar_tensor_tensor(
                out=out_t[:], in0=pos_sb[:], scalar=-M_SHIFT, in1=Lt[:],
                op0=Add, op1=Sub,
            )
            nc.sync.dma_start(out_ap[:], out_t[:])

    _split_multi_waits(nc)
    return nc


def _core_blocks(c):
    """Per-core mm1 blocks: 3 full pairs + 1 half pair, uniform program.

    Global combo g = pair*8 + j; core c owns combos [28c, 28c+28).
    Returns ([(pair, j_start, n_j)] * 4) with the half block last.
    """
    if c % 2 == 0:
        p0 = (7 * c) // 2
        return [(p0, 0, 8), (p0 + 1, 0, 8), (p0 + 2, 0, 8), (p0 + 3, 0, 4)]
    else:
        p0 = (7 * c) // 2
        return [(p0 + 1, 0, 8), (p0 + 2, 0, 8), (p0 + 3, 0, 8), (p0, 4, 4)]


def _prep_inputs(Z, C, Wk, bk):
    ii, kk = np.triu_indices(H, 1)

    # Zneg columns n = (h*8+w)*64 + b, rows d; DR layout [p, q, g, i, c]
    Znegs = Z.transpose(1, 2, 3, 0).reshape(D, 4096) / WSCALE
    zn = Znegs.reshape(2, 2, 128, 4, 1024).transpose(2, 3, 0, 1, 4)
    zn = np.ascontiguousarray(zn).astype(NPF8)

    # W^T in DR layout per pair: wdrp[pair][p, g2, i2, ec, f]
    # = Wk[k-1][128ec+f, 256g2+128i2+p] * WSCALE
    WT = (Wk.transpose(0, 2, 1) * WSCALE)  # [pair_k][d, e]
    wdr_all = WT.reshape(7, 2, 2, 128, 4, 128).transpose(0, 3, 1, 2, 4, 5)
    wdr_all = np.ascontiguousarray(wdr_all).astype(NPF8)  # [7, p, g2, i2, ec, f]

    # C^T per (i, j): [d, b] ; zpos per (k, j): [d, b]
    Ctr = C.transpose(2, 3, 1, 0)  # [i, j, d, b]
    Ztr = Z.transpose(2, 3, 1, 0)  # [h, w, d, b]

    m64 = (np.arange(64)[None, :] != np.arange(128)[:, None] % 64)
    m64 = m64.astype(BF16)
    eye = np.eye(128, dtype=np.float32)

    in_maps = []
    for c in range(NCORES):
        blocks = _core_blocks(c)
        wdr = np.empty((NBLK, 128, 2, 2, 4, 128), NPF8)
        cdr = np.zeros((NBLK, 128, 2, 2, 512), NPF8)
        bgc = np.empty((NBLK, 128, 4), np.float32)
        zpc = np.empty((NCHUNKS, 128, 4, 128), NPF8)
        t = 0
        for blk, (pair, j0, nj) in enumerate(blocks):
            i_, k_ = int(ii[pair]), int(kk[pair])
            wdr[blk] = wdr_all[k_ - 1]
            bgc[blk] = bk[k_ - 1].reshape(4, 128).T * WSCALE
            # cdr[p, g2, i2, r] with r = (j-j0)*64 + b, d = 256g2+128i2+p
            cblk = Ctr[i_, j0:j0 + nj]          # [nj, d, b]
            cblk = cblk.transpose(1, 0, 2).reshape(2, 2, 128, nj * 64)
            cdr[blk, :, :, :, 0:nj * 64] = cblk.transpose(2, 0, 1, 3).astype(NPF8)
            # chunks of this block
            for tb in range(nj // 2):
                ja, jb_ = j0 + 2 * tb, j0 + 2 * tb + 1
                zp = np.concatenate(
                    [Ztr[k_, ja], Ztr[k_, jb_]], axis=1) / WSCALE  # [d, 128]
                zpc[t] = zp.reshape(4, 128, 128).transpose(1, 0, 2).astype(NPF8)
                t += 1
        assert t == NCHUNKS
        in_maps.append({
            "zn": zn, "wdr": wdr, "cdr": cdr, "bgc": bgc, "zpc": zpc,
            "m64": m64, "eye": eye,
        })
    return in_maps


def _chunk_info(c):
    """Host-side (k, j0) per chunk for each core (for testing)."""
    ii, kk = np.triu_indices(H, 1)
    info = []
    for pair, j0, nj in _core_blocks(c):
        for tb in range(nj // 2):
            info.append((int(ii[pair]), int(kk[pair]), j0 + 2 * tb))
    return info


def kernel(Z, C, Wk, bk):
    global LAST_RESULTS
    Z = np.asarray(Z, np.float32)
    C = np.asarray(C, np.float32)
    Wk = np.asarray(Wk, np.float32)
    bk = np.asarray(bk, np.float32)

    if "nc" not in _cache:
        _cache["nc"] = _build_module()
    nc = _cache["nc"]

    in_maps = _prep_inputs(Z, C, Wk, bk)
    res = run_bass_kernel_spmd(nc, in_maps, core_ids=list(range(NCORES)))
    LAST_RESULTS = res
    total = np.float64(0.0)
    for c in range(NCORES):
        total += np.sum(res.results[c]["out"].astype(np.float64))
    loss = -(total / (NCORES * NCHUNKS * 128))
    return np.array(loss, dtype=np.float32)
